# revision 1
# baseline (speedup 1.0000x reference)
"""Correlation kernel for Trainium2 (Bass/Tile), 8 NeuronCores.

Problem: inputs (B=4, N=2, C=128, H=128, W=128) fp32.
  src = inputs[:, 0], target = inputs[:, 1]
  out[b, k, y, x] = (1/C) * sum_c src[b,c,y,x] * target[b,c,y+dy,x+dx]
  for k = (dy+10)*21 + (dx+10), dy,dx in [-10,10], zero-padded target.
  Output (4, 441, 128, 128) fp32.

Mapping:
  - Shard over 8 cores: (b in 0..3) x (H half in 0..1). Each core handles
    64 output rows; halos come from host-side padded slabs.
  - Per output row y, contraction over C runs on the PE:
      stationary = src row chunk (128c x 32x), col-tiled at tile_position
      (0, 32g) so 4 x-chunks share the array;
      moving = padded target rows [y+dy', x-window 52 wide], 7 dy per
      matmul (N = 7*52 = 364 <= 512, one PSUM bank).
    PSUM tile per y: [128, 3, 512] (3 banks).
    Numerics: inputs are split on the host into bf16 hi + lo halves and
    the product is computed as hi*hi + hi*lo + lo*hi (3 accumulating
    matmuls, ~1e-5 rel err); bf16 is the fast/safe PE path.
  - DVE/ScalarE evacuate PSUM -> SBUF; 4 rows are batched per output DMA
    (2.2 MB transfers) into a (64,128,1092) per-core "window" tensor.
  - Host extracts the 21 needed diagonals per 52-wide window
    (out[..., dx] = win[..., (x mod 32) + dx]) while unsharding.
Scaling by 1/C is folded into src on the host (exact: 2^-7).
"""

import os

import ml_dtypes
import numpy as np

import concourse.bacc as bacc
import concourse.bass as bass
import concourse.mybir as mybir
import concourse.tile as tile
from concourse.bass_utils import run_bass_kernel_spmd

B = 4
C = 128
H = 128
W = 128
KS = 21          # kernel size (per axis)
P = KS // 2      # pad / max displacement = 10
HY = H // 2      # rows per core = 64
NG = 4           # x groups (col-tiling), 32 wide each
GW = 32          # group width
WIN = GW + 2 * P     # 52: target x-window per group
DYB = 3          # dy batches
DYI = KS // DYB  # 7 dy per batch
NMOV = DYI * WIN     # 364 moving columns per matmul
TGT_H = HY + 2 * P   # 84 target rows per core
TGT_W = W + 2 * P    # 148 padded target width
OUTF = DYB * NMOV    # 1092 values per (y, x)
YB = 4               # output rows per store DMA

_CACHE = {}


def _build_module(mode: str):
    """Build the SPMD Bass module (same program on all 8 cores)."""
    f32 = mybir.dt.float32
    bf16 = mybir.dt.bfloat16
    nc = bacc.Bacc("TRN2", target_bir_lowering=False, debug=False)

    split = mode.startswith("bf16")
    in_dt = bf16 if split else f32
    src_names = ["src_hi", "src_lo"] if mode == "bf16x3" else ["src_hi"]
    tgt_names = ["tgt_hi", "tgt_lo"] if mode == "bf16x3" else ["tgt_hi"]

    src_d = {n: nc.declare_dram_parameter(n, [C, HY, W], in_dt, isOutput=False)
             for n in src_names}
    tgt_d = {n: nc.declare_dram_parameter(n, [C, TGT_H, TGT_W], in_dt, isOutput=False)
             for n in tgt_names}
    out_d = nc.declare_dram_parameter("out_win", [HY, 128, OUTF], f32, isOutput=True)

    mm_dt = mybir.dt.float32r if mode == "fp32r" else in_dt

    with tile.TileContext(nc) as tc:
        with (
            tc.tile_pool(name="inp", bufs=1) as inp,
            tc.tile_pool(name="psum", bufs=2, space=bass.MemorySpace.PSUM) as psum,
            tc.tile_pool(name="win", bufs=4) as winp,
        ):
            src_sb = {n: inp.tile([C, HY, W], in_dt, name=f"sb_{n}")
                      for n in src_names}
            tgt_sb = {n: inp.tile([C, TGT_H, TGT_W], in_dt, name=f"sb_{n}")
                      for n in tgt_names}
            # Split loads so early rows' matmuls can start before the whole
            # slab lands.
            nchunk = 8
            for i in range(nchunk):
                ys = (TGT_H + nchunk - 1) // nchunk
                lo = i * ys
                hi = min(TGT_H, lo + ys)
                for n in tgt_names:
                    nc.sync.dma_start(tgt_sb[n][:, lo:hi, :], tgt_d[n][:, lo:hi, :])
                ys = (HY + nchunk - 1) // nchunk
                lo = i * ys
                hi = min(HY, lo + ys)
                for n in src_names:
                    nc.sync.dma_start(src_sb[n][:, lo:hi, :], src_d[n][:, lo:hi, :])

            if mode == "bf16x3":
                passes = [("src_hi", "tgt_hi"), ("src_hi", "tgt_lo"),
                          ("src_lo", "tgt_hi")]
            else:
                passes = [("src_hi", "tgt_hi")]

            for yb in range(HY // YB):
                win = winp.tile([128, YB, DYB, NMOV], f32)
                for yy in range(YB):
                    y = yb * YB + yy
                    ps = psum.tile([128, DYB, 512], f32)
                    # pass-major inside each dy batch: consecutive matmuls
                    # hit different col-strips, so LDWEIGHTS prefetch hides
                    # behind the previous strip's matmul.
                    for dyb in range(DYB):
                        for ip, (sn, tn) in enumerate(passes):
                            for g in range(NG):
                                lhsT = src_sb[sn][:, y, g * GW:(g + 1) * GW]
                                rhs = tgt_sb[tn][:, y + dyb * DYI:
                                                 y + (dyb + 1) * DYI,
                                                 g * GW: g * GW + WIN]
                                nc.tensor.matmul(
                                    ps[g * GW:(g + 1) * GW, dyb, 0:NMOV],
                                    lhsT.bitcast(mm_dt),
                                    rhs.bitcast(mm_dt),
                                    start=(ip == 0),
                                    stop=(ip == len(passes) - 1),
                                    tile_position=(0, g * GW),
                                )
                    if y % 2 == 0:
                        nc.vector.tensor_copy(win[:, yy], ps[:, :, 0:NMOV])
                    else:
                        nc.scalar.copy(win[:, yy], ps[:, :, 0:NMOV])
                nc.sync.dma_start(
                    out_d[yb * YB:(yb + 1) * YB].rearrange("y p f -> p y f"),
                    win[:].rearrange("p y a b -> p y (a b)"),
                )

    nc.compile()
    return nc


def _get_module(mode: str):
    if mode not in _CACHE:
        _CACHE[mode] = _build_module(mode)
    return _CACHE[mode]


def _split_bf16(x):
    hi = x.astype(ml_dtypes.bfloat16)
    lo = (x - hi.astype(np.float32)).astype(ml_dtypes.bfloat16)
    return hi, lo


def _shard_inputs(inputs: np.ndarray, mode: str):
    src = np.ascontiguousarray(inputs[:, 0]) * np.float32(1.0 / C)  # exact
    tgt = inputs[:, 1]
    tgt_pad = np.pad(tgt, ((0, 0), (0, 0), (P, P), (P, P)))
    in_maps = []
    for core in range(8):
        b, h = divmod(core, 2)
        s = np.ascontiguousarray(src[b, :, h * HY:(h + 1) * HY, :])
        t = np.ascontiguousarray(tgt_pad[b, :, h * HY: h * HY + TGT_H, :])
        if mode.startswith("bf16"):
            s_hi, s_lo = _split_bf16(s)
            t_hi, t_lo = _split_bf16(t)
            m = {"src_hi": s_hi, "tgt_hi": t_hi}
            if mode == "bf16x3":
                m["src_lo"] = s_lo
                m["tgt_lo"] = t_lo
        else:
            m = {"src_hi": s, "tgt_hi": t}
        in_maps.append(m)
    return in_maps


# (x mod 32) + dx' index into the 52-wide window, for each (x, dx')
_XIDX = (np.arange(128) % GW)[:, None] + np.arange(KS)[None, :]  # (128, 21)


def _extract(win: np.ndarray) -> np.ndarray:
    """(HY, 128, OUTF) window tensor -> (441, HY, 128) output block."""
    w4 = win.reshape(HY, 128, KS, WIN)  # [y, x, dy', u]
    idx = np.broadcast_to(_XIDX[None, :, None, :], (HY, 128, KS, KS))
    o4 = np.take_along_axis(w4, idx, axis=3)  # [y, x, dy', dx']
    return o4.transpose(2, 3, 0, 1).reshape(KS * KS, HY, 128)


def run(inputs: np.ndarray, trace: bool = False, mode: str | None = None):
    if mode is None:
        mode = os.environ.get("CORR_MM_MODE", "bf16x3")
    nc = _get_module(mode)
    in_maps = _shard_inputs(inputs, mode)
    res = run_bass_kernel_spmd(
        nc, in_maps, core_ids=list(range(8)), trace=trace,
    )
    out = np.empty((B, KS * KS, H, W), dtype=np.float32)
    for core in range(8):
        b, h = divmod(core, 2)
        out[b, :, h * HY:(h + 1) * HY, :] = _extract(res.results[core]["out_win"])
    return out, res.exec_time_ns


def kernel(inputs: np.ndarray) -> np.ndarray:
    out, _ = run(np.asarray(inputs))
    return out



# revision 7
# speedup vs baseline: 2.1487x; 2.1487x over previous
"""Correlation kernel for Trainium2 (Bass/Tile), 8 NeuronCores.

Problem: inputs (B=4, N=2, C=128, H=128, W=128) fp32.
  src = inputs[:, 0], target = inputs[:, 1]
  out[b, k, y, x] = (1/C) * sum_c src[b,c,y,x] * target[b,c,y+dy,x+dx]
  for k = (dy+10)*21 + (dx+10), dy,dx in [-10,10], zero-padded target.
  Output (4, 441, 128, 128) fp32.

Mapping (v2, 2D-patch matmuls):
  - Shard over 8 cores: (b in 0..3) x (H half in 0..1); 64 rows/core.
  - Per core, pixels are tiled into 64 patches of 16(y) x 8(x) = 128
    pixels. One patch = one stationary lhsT (C=128 x 128 pixels, full PE
    array). The moving rhs is the target window for the whole patch:
    36 rows (16+2*10) x 28 cols (8+2*10) = 1008 columns, split into two
    N=504 matmuls (one PSUM bank each). Each pixel's row of the matmul
    output holds its full 36x28 window of correlation values; the host
    extracts the needed 21x21 block per pixel while unsharding.
  - Everything is fp16: inputs are pre-scaled on the host by 2^-4/2^-3
    (exact; folds in the 1/C=2^-7 mean) and cast to fp16 (~5e-4 rel
    err); PE accumulates in fp32; PSUM->SBUF evacuation (DVE/ACT
    alternating) downcasts to fp16 for the window DMA. Total DMA is
    ~21.7 MB/core vs 46.6 MB for the previous strip-mined version,
    and PE streaming columns drop 4.3x (1008 cols / 128-px patch).
"""

import numpy as np

import concourse.bacc as bacc
import concourse.bass as bass
import concourse.mybir as mybir
import concourse.tile as tile
from concourse.bass_utils import run_bass_kernel_spmd

B = 4
C = 128
H = 128
W = 128
KS = 21          # kernel size (per axis)
P = KS // 2      # pad / max displacement = 10
HY = H // 2      # rows per core = 64
PY = 16          # patch rows
PX = 8           # patch cols (PY*PX = 128 = M)
TH = PY + 2 * P  # 36: target row window per patch
XW = PX + 2 * P  # 28: target col window per patch
NBY = HY // PY   # 4 bands
NBX = W // PX    # 16 x-chunks
NPATCH = NBY * NBX   # 64 patches per core
WINF = TH * XW       # 1008 window values per pixel
NSPL = 2             # matmul N-split (504 <= 512 psum bank)
TSPL = TH // NSPL    # 18 t-rows per matmul
TGT_H = HY + 2 * P   # 84 target rows per core
TGT_W = W + 2 * P    # 148 padded target width
KB = 4               # patches per output DMA (v1)
NQ = 4               # py-quads per band (v2): 4 py rows = 32 partitions
QPY = PY // NQ       # 4 py rows per quad
TQ = KS + QPY - 1    # 24: t-rows shipped per quad (union of 4 pixels' 21)

_CACHE = {}


def _build_module(mode: str):
    """Build the SPMD Bass module (same program on all 8 cores)."""
    f32 = mybir.dt.float32
    f16 = mybir.dt.float16
    nc = bacc.Bacc("TRN2", target_bir_lowering=False, debug=False)

    # src is pre-tiled on the host to [C, patch, pixel] so each patch's
    # 128 pixels are one contiguous free dim (stationary APs must be 1D)
    src_d = nc.declare_dram_parameter("src", [C, NPATCH, PY * PX], f16,
                                      isOutput=False)
    tgt_d = nc.declare_dram_parameter("tgt", [C, TGT_H, TGT_W], f16,
                                      isOutput=False)
    out_d = nc.declare_dram_parameter("out_win", [NPATCH, 128, WINF], f16,
                                      isOutput=True)

    with tile.TileContext(nc) as tc:
        with (
            tc.tile_pool(name="inp", bufs=1) as inp,
            tc.tile_pool(name="psum", bufs=3, space=bass.MemorySpace.PSUM) as psum,
            tc.tile_pool(name="win", bufs=4) as winp,
        ):
            src_sb = inp.tile([C, NPATCH, PY * PX], f16, name="sb_src")
            tgt_sb = inp.tile([C, TGT_H, TGT_W], f16, name="sb_tgt")
            # Chunked loads so band 0's matmuls can start early.
            tgt_rows = [(0, 18), (18, 36), (36, 52), (52, 68), (68, 84)]
            src_rows = [(0, 16), (16, 32), (32, 48), (48, 64)]  # patch idx
            nc.sync.dma_start(tgt_sb[:, 0:18, :], tgt_d[:, 0:18, :])
            nc.sync.dma_start(tgt_sb[:, 18:36, :], tgt_d[:, 18:36, :])
            nc.sync.dma_start(src_sb[:, 0:16, :], src_d[:, 0:16, :])
            for (lo, hi), (slo, shi) in zip(tgt_rows[2:], src_rows[1:]):
                nc.sync.dma_start(tgt_sb[:, lo:hi, :], tgt_d[:, lo:hi, :])
                nc.sync.dma_start(src_sb[:, slo:shi, :], src_d[:, slo:shi, :])

            for by in range(NBY):
                for g in range(NBX // KB):
                    win = winp.tile([128, KB, WINF], f16)
                    for j in range(KB):
                        bx = g * KB + j
                        p = by * NBX + bx
                        ps = psum.tile([128, NSPL, 512], f32)
                        lhsT = src_sb[:, p, :]
                        for k in range(NSPL):
                            rhs = tgt_sb[:, by * PY + k * TSPL:
                                         by * PY + (k + 1) * TSPL,
                                         bx * PX: bx * PX + XW]
                            nc.tensor.matmul(
                                ps[:, k, 0:TSPL * XW],
                                lhsT, rhs, start=True, stop=True,
                            )
                        if p % 2 == 0:
                            nc.vector.tensor_copy(win[:, j], ps[:, :, 0:TSPL * XW])
                        else:
                            nc.scalar.copy(win[:, j], ps[:, :, 0:TSPL * XW])
                    p0 = by * NBX + g * KB
                    nc.sync.dma_start(
                        out_d[p0:p0 + KB].rearrange("k p f -> p k f"),
                        win[:],
                    )

    nc.compile()
    return nc


def _get_module(mode: str):
    if mode not in _CACHE:
        _CACHE[mode] = _build_module(mode)
    return _CACHE[mode]


def _shard_inputs(inputs: np.ndarray, mode: str):
    # fold the 1/C = 2^-7 mean into the inputs as 2^-3 * 2^-4 (exact,
    # and keeps both operands well inside fp16 normal range)
    src = (inputs[:, 0] * np.float32(0.125)).astype(np.float16)
    tgt = (inputs[:, 1] * np.float32(0.0625)).astype(np.float16)
    tgt_pad = np.pad(tgt, ((0, 0), (0, 0), (P, P), (P, P)))
    in_maps = []
    for core in range(8):
        b, h = divmod(core, 2)
        s = src[b, :, h * HY:(h + 1) * HY, :]
        # pre-tile to [C, patch=(by,bx), pixel=(py,px)]
        s = (s.reshape(C, NBY, PY, NBX, PX).transpose(0, 1, 3, 2, 4)
             .reshape(C, NPATCH, PY * PX))
        s = np.ascontiguousarray(s)
        t = np.ascontiguousarray(tgt_pad[b, :, h * HY: h * HY + TGT_H, :])
        in_maps.append({"src": s, "tgt": t})
    return in_maps


# flat window index for pixel m=(py,px) and displacement (dy,dx):
# (py+dy+10)*XW + (px+dx+10)  -> shape (128, 441)
_pyv = np.arange(PY)
_pxv = np.arange(PX)
_dv = np.arange(KS)
_FLAT = ((_pyv[:, None, None, None] + _dv[None, None, :, None]) * XW
         + _pxv[None, :, None, None] + _dv[None, None, None, :]
         ).reshape(128, KS * KS)


def _extract(win: np.ndarray) -> np.ndarray:
    """(NPATCH, 128, WINF) window tensor -> (441, HY, W) output block."""
    g = np.take_along_axis(win, _FLAT[None, :, :], axis=2)  # (64, 128, 441)
    arr = g.reshape(NBY, NBX, PY, PX, KS * KS)
    return (arr.transpose(4, 0, 2, 1, 3)
            .reshape(KS * KS, HY, W).astype(np.float32))


def run(inputs: np.ndarray, trace: bool = False, mode: str | None = None):
    mode = "p16x8"
    nc = _get_module(mode)
    in_maps = _shard_inputs(inputs, mode)
    res = run_bass_kernel_spmd(
        nc, in_maps, core_ids=list(range(8)), trace=trace,
    )
    out = np.empty((B, KS * KS, H, W), dtype=np.float32)
    for core in range(8):
        b, h = divmod(core, 2)
        out[b, :, h * HY:(h + 1) * HY, :] = _extract(res.results[core]["out_win"])
    return out, res.exec_time_ns


def kernel(inputs: np.ndarray) -> np.ndarray:
    out, _ = run(np.asarray(inputs))
    return out
